# revision 19
# baseline (speedup 1.0000x reference)
"""Trainium2 Bass kernel for a ViT-style transformer block (sparse_attention).

v3: fp8 (e4m3) DoubleRow matmuls for qkv/pv/fc1/fc2 (K=256 per instruction,
2x+ PE throughput), bf16 scores/proj. Data-parallel over batch B=32 across
8 cores (4 items/core), no collectives.

Key structure per item:
  LN1 (DVE stats + Newton-rsqrt + apply) -> PE transpose -> hT fp8 (t-major
  [128, 5t, 6kc, 128] so DoubleRow stationary pairs are contiguous).
  q/k feature-major bf16 via fp8 DR matmuls (weights x256, descale 2^-8 in
  the PSUM->SBUF writeback); v -> fp8 vpe/vpo with ones-columns for softmax
  denominators. Scores via K=64 matmuls at base partition 0/64 (no zero
  padding); mask-mult split DVE/Pool; exp on ACT -> fp8; PV via DR pairs.
  Denominators broadcast per head-pair via a K=2 selector matmul, fast
  reciprocal, normalize on Pool -> attn bf16. proj bf16 + residual (Pool),
  LN2, fc1 DR + Gelu (descale via ACT scale) -> fp8, fc2 DR + fused
  descale+residual (scalar_tensor_tensor) -> out.

Work of item i-1 (proj/LN2/fc1/fc2) is interleaved into item i's head loop
to keep all engines busy (software pipeline from v2).
"""

import sys

sys.path.insert(0, "/opt/trn_rl_repo")

import numpy as np
import ml_dtypes

import concourse.bass as bass
import concourse.tile as tile
from concourse import bacc, mybir
from concourse import bass_utils
from concourse.masks import make_identity

F32 = mybir.dt.float32
BF16 = mybir.dt.bfloat16
F8 = mybir.dt.float8e4
I32 = mybir.dt.int32
DR = mybir.MatmulPerfMode.DoubleRow

B = 32
N = 577
D = 768
H = 12
DH = 64
HID = 3072
DCH = D // 128          # 6 chunks of the model dim
HCH = HID // 128        # 24 chunks of the hidden dim
NCORES = 8
IPC = B // NCORES       # items per core
TOK = IPC * N           # tokens per core

NT = [(0, 128), (128, 128), (256, 128), (384, 128), (512, 65)]
SPL_N = [(0, 512), (512, 65)]
SPL_D = [(0, 512), (512, 256)]
EPS = 1e-5
WS = 256.0              # fp8 weight scale
IWS = 1.0 / WS

AF = mybir.ActivationFunctionType
ALU = mybir.AluOpType


def build_nc(use_bias_mm=True):
    nc = bacc.Bacc("TRN2", target_bir_lowering=False, debug=False, num_devices=NCORES)

    x_d = nc.dram_tensor("x", [TOK, D], F32, kind="ExternalInput").ap()
    maskt_d = nc.dram_tensor("maskt", [N, N], BF16, kind="ExternalInput").ap()
    # wq/wk: [128, mc 6, kc 6, 128] fp8 (x256); wv: [128, kc 6, 768] fp8 (x256)
    wq_d = nc.dram_tensor("wq", [128, DCH, DCH, 128], F8, kind="ExternalInput").ap()
    wk_d = nc.dram_tensor("wk", [128, DCH, DCH, 128], F8, kind="ExternalInput").ap()
    wv_d = nc.dram_tensor("wv", [128, DCH, D], F8, kind="ExternalInput").ap()
    bq_d = nc.dram_tensor("bq", [D], F32, kind="ExternalInput").ap()
    bk_d = nc.dram_tensor("bk", [D], F32, kind="ExternalInput").ap()
    wproj_d = nc.dram_tensor("wproj", [128, DCH, D], BF16, kind="ExternalInput").ap()
    wfc1_d = nc.dram_tensor("wfc1", [128, HCH, DCH, 128], F8, kind="ExternalInput").ap()
    bfc1_d = nc.dram_tensor("bfc1", [HID], F32, kind="ExternalInput").ap()
    wfc2_d = nc.dram_tensor("wfc2", [128, HCH, D], F8, kind="ExternalInput").ap()
    if use_bias_mm:
        bprojr_d = nc.dram_tensor("bprojr", [D], BF16, kind="ExternalInput").ap()
        bfc2r_d = nc.dram_tensor("bfc2r", [D], BF16, kind="ExternalInput").ap()
    out_d = nc.dram_tensor("out", [TOK, D], F32, kind="ExternalOutput").ap()

    with tile.TileContext(nc) as tc:
        with (
            tc.tile_pool(name="const", bufs=1) as const,
            tc.tile_pool(name="work", bufs=1) as work,
            tc.tile_pool(name="psum", bufs=1, space="PSUM") as psum,
        ):
            # ---- constants / weights (resident) ----
            wq_sb = const.tile([128, DCH, DCH, 128], F8, name="wq_sb")
            nc.sync.dma_start(out=wq_sb, in_=wq_d)
            wk_sb = const.tile([128, DCH, DCH, 128], F8, name="wk_sb")
            nc.sync.dma_start(out=wk_sb, in_=wk_d)
            wv_sb = const.tile([128, DCH, D], F8, name="wv_sb")
            nc.sync.dma_start(out=wv_sb, in_=wv_d)
            wproj_sb = const.tile([128, DCH, D], BF16, name="wproj_sb")
            wfc1_sb = const.tile([128, HCH, DCH, 128], F8, name="wfc1_sb")
            wfc2_sb = const.tile([128, HCH, D], F8, name="wfc2_sb")

            bq_sb = const.tile([128, DCH], F32, name="bq_sb")
            nc.sync.dma_start(out=bq_sb, in_=bq_d.rearrange("(c p) -> p c", p=128))
            bk_sb = const.tile([128, DCH], F32, name="bk_sb")
            nc.sync.dma_start(out=bk_sb, in_=bk_d.rearrange("(c p) -> p c", p=128))
            bfc1_sb = const.tile([128, HCH], F32, name="bfc1_sb")
            nc.sync.dma_start(out=bfc1_sb, in_=bfc1_d.rearrange("(c p) -> p c", p=128))
            if use_bias_mm:
                bprojr_sb = const.tile([1, D], BF16, name="bprojr_sb")
                nc.sync.dma_start(out=bprojr_sb, in_=bprojr_d[None, :])
                bfc2r_sb = const.tile([1, D], BF16, name="bfc2r_sb")
                nc.sync.dma_start(out=bfc2r_sb, in_=bfc2r_d[None, :])
                ones_row = const.tile([1, N], BF16, name="ones_row")
                nc.vector.memset(ones_row, 1.0)

            maskt_sb = const.tile([128, 5, N], BF16, name="maskt_sb")
            nc.gpsimd.memset(maskt_sb[:, 4, :], 0.0)
            for mt, (mo, msz) in enumerate(NT):
                nc.sync.dma_start(out=maskt_sb[:msz, mt, :], in_=maskt_d[mo:mo + msz, :])

            ident = const.tile([128, 128], BF16, name="ident")
            make_identity(nc, ident)
            # selector rows for pair-denominator broadcast. The even-head
            # denominator sits on partition 64 (ones col 64), the odd-head
            # one on partition 0; each selector row lives on the matching
            # partition: slot 0 @ p64 spreads even -> partitions 0:64,
            # slot 1 @ p0 spreads odd -> partitions 64:128.
            selmat = const.tile([128, 2, 128], BF16, name="selmat")
            nc.gpsimd.memset(selmat[0:1], 0.0)
            nc.gpsimd.memset(selmat[64:65], 0.0)
            nc.gpsimd.memset(selmat[64:65, 0, 0:64], 1.0)
            nc.gpsimd.memset(selmat[0:1, 1, 64:128], 1.0)

            # v stationary tiles [128 keys, c 6, mt 5, 128] fp8
            # vpe: cols 0:64 = even-head dims, col 64 = ones (denom), 65:128 = 0
            # vpo: col 0 = ones (denom), 1:64 = 0, cols 64:128 = odd-head dims
            vpe = const.tile([128, DCH, 5, 128], F8, name="vpe")
            vpo = const.tile([128, DCH, 5, 128], F8, name="vpo")
            nc.gpsimd.memset(vpe, 0.0)
            nc.gpsimd.memset(vpo, 0.0)
            for mt, (mo, msz) in enumerate(NT):
                nc.gpsimd.memset(vpe[0:msz, :, mt, 64:65], 1.0)
                nc.gpsimd.memset(vpo[0:msz, :, mt, 0:1], 1.0)

            # persistent per-item buffers (stable addresses via fixed tags)
            k_sb = const.tile([128, DCH, 640], BF16, name="k_sb")
            nc.gpsimd.memset(k_sb, 0.0)
            q_sb = const.tile([128, DCH, N], BF16, name="q_sb")
            hT_t = [const.tile([128, 5, DCH, 128], F8, name=f"hT{i}") for i in range(2)]
            for t_ in hT_t:
                nc.gpsimd.memset(t_[:, 4, :, 65:128], 0.0)
            h2T = const.tile([128, 5, DCH, 128], F8, name="h2T")
            nc.gpsimd.memset(h2T[:, 4, :, 65:128], 0.0)
            g2 = const.tile([128, 5, HCH, 128], F8, name="g2")

            def newton_rsqrt(var_ap, rstd, pfx):
                """rstd[:,0:5] = (var_ap + EPS) ** -0.5, DVE only."""
                ve = work.tile([128, 5], F32, name=f"ve_{pfx}", tag="nwt_ve", bufs=2)
                nc.vector.tensor_scalar(out=ve, in0=var_ap, scalar1=EPS, scalar2=None,
                                        op0=ALU.add)
                yi = work.tile([128, 5], I32, name=f"yi_{pfx}", tag="nwt_yi", bufs=2)
                nc.vector.tensor_scalar(out=yi, in0=ve.bitcast(I32), scalar1=1,
                                        scalar2=None, op0=ALU.logical_shift_right)
                nc.vector.tensor_scalar(out=yi, in0=yi, scalar1=-1, scalar2=None,
                                        op0=ALU.bitwise_xor)
                nc.vector.tensor_scalar(out=yi, in0=yi, scalar1=0x5f3759e0,
                                        scalar2=None, op0=ALU.add)
                y = yi.bitcast(F32)
                t1 = work.tile([128, 5], F32, name=f"t1_{pfx}", tag="nwt_t1", bufs=2)
                for _ in range(2):
                    nc.vector.tensor_tensor(out=t1, in0=y, in1=y, op=ALU.mult)
                    nc.vector.tensor_tensor(out=t1, in0=t1, in1=ve, op=ALU.mult)
                    nc.vector.tensor_scalar(out=t1, in0=t1, scalar1=-0.5, scalar2=1.5,
                                            op0=ALU.mult, op1=ALU.add)
                    nc.vector.tensor_tensor(out=y, in0=y, in1=t1, op=ALU.mult)
                nc.vector.tensor_copy(out=rstd, in_=y)

            def ln_stats(src, statsall, mvall, t, tsz, pfx):
                nc.vector.bn_stats(out=statsall[:tsz, t, 0, :], in_=src[:tsz, 0:256])
                nc.vector.bn_stats(out=statsall[:tsz, t, 1, :], in_=src[:tsz, 256:512])
                nc.vector.bn_stats(out=statsall[:tsz, t, 2, :], in_=src[:tsz, 512:768])
                nc.vector.bn_aggr(out=mvall[:tsz, t, :], in_=statsall[:tsz, t])

            def ln_apply_tp(src, mvall, rstd, dst, t, tsz, pfx, cp_eng):
                """(src - mean) * rstd -> bf16 -> PE transpose -> dst[:, t] fp8."""
                htm = work.tile([128, D], BF16, name=f"htm_{pfx}_{t}", tag="htm", bufs=2)
                nc.gpsimd.tensor_scalar(out=htm[:tsz], in0=src[:tsz],
                                        scalar1=mvall[:tsz, t, 0:1],
                                        scalar2=rstd[:tsz, t:t + 1],
                                        op0=ALU.subtract, op1=ALU.mult)
                tp = psum.tile([128, D], BF16, name=f"tp_{pfx}_{t}", tag="small", bufs=2)
                for c in range(DCH):
                    nc.tensor.transpose(tp[:, c * 128:c * 128 + tsz],
                                        htm[:tsz, c * 128:(c + 1) * 128],
                                        ident[:tsz, :tsz])
                dst_ap = bass.AP(tensor=dst.tensor, offset=dst.offset + t * DCH * 128,
                                 ap=[dst.ap[0], [128, DCH], [1, tsz]])
                src_ap = tp.rearrange("p (c q) -> p c q", c=DCH)[:, :, :tsz]
                if cp_eng == "act":
                    nc.scalar.activation(out=dst_ap, in_=src_ap, func=AF.Copy)
                else:
                    nc.vector.tensor_copy(out=dst_ap, in_=src_ap)

            def emit_A(it):
                """x load, LN1, hT fp8, q/k bf16, v fp8 for item `it`."""
                t0 = it * N
                hT = hT_t[it % 2]
                xall = work.tile([128, 5, D], F32, name=f"xall_{it}", tag="xall", bufs=1)
                statsall = work.tile([128, 5, 3, 6], F32, name=f"st1_{it}",
                                     tag="stats1", bufs=1)
                mvall = work.tile([128, 5, 2], F32, name=f"mv1_{it}", tag="mv1", bufs=1)
                rstd = work.tile([128, 5], F32, name=f"rs1_{it}", tag="rstd1", bufs=1)
                for t, (o, tsz) in enumerate(NT):
                    nc.sync.dma_start(out=xall[:tsz, t, :], in_=x_d[t0 + o:t0 + o + tsz, :])
                    ln_stats(xall[:, t, :], statsall, mvall, t, tsz, f"a{it}")
                newton_rsqrt(mvall[:, :, 1], rstd, f"a{it}")
                for t, (o, tsz) in enumerate(NT):
                    ln_apply_tp(xall[:, t, :], mvall, rstd, hT, t, tsz, f"a{it}", "act")

                # rhs moving views per pair: t0..t3 (512 cols) + t4 (128 cols)
                def hT_rhs03(hh, p):
                    return bass.AP(tensor=hh.tensor, offset=hh.offset + 2 * p * 128,
                                   ap=[hh.ap[0], [128, 2], [DCH * 128, 4], [1, 128]])

                def qk_chunk(ps, w_sb_, mc):
                    for p in range(3):
                        nc.tensor.matmul(ps[:, 0:512], w_sb_[:, mc, 2 * p:2 * p + 2, :],
                                         hT_rhs03(hT, p), start=(p == 0), stop=(p == 2),
                                         perf_mode=DR)
                    for p in range(3):
                        nc.tensor.matmul(ps[:, 512:640], w_sb_[:, mc, 2 * p:2 * p + 2, :],
                                         hT[:, 4, 2 * p:2 * p + 2, :],
                                         start=(p == 0), stop=(p == 2), perf_mode=DR)

                def wb(dst, ps, b_sb, mc):
                    if use_bias_mm:
                        nc.vector.tensor_scalar(out=dst, in0=ps[:, 0:N], scalar1=IWS,
                                                scalar2=b_sb[:, mc:mc + 1],
                                                op0=ALU.mult, op1=ALU.add)
                    else:
                        nc.vector.tensor_scalar(out=dst, in0=ps[:, 0:N], scalar1=IWS,
                                                scalar2=None, op0=ALU.mult)

                for mc in range(DCH):
                    ps = psum.tile([128, D], F32, name=f"psq_{it}_{mc}", tag="big", bufs=3)
                    qk_chunk(ps, wq_sb, mc)
                    wb(q_sb[:, mc, :], ps, bq_sb, mc)
                for mc in range(DCH):
                    ps = psum.tile([128, D], F32, name=f"psk_{it}_{mc}", tag="big", bufs=3)
                    qk_chunk(ps, wk_sb, mc)
                    wb(k_sb[:, mc, 0:N], ps, bk_sb, mc)
                for t, (o, tsz) in enumerate(NT):
                    ps = psum.tile([128, D], F32, name=f"psv_{it}_{t}", tag="big", bufs=3)
                    for p in range(3):
                        for (o2, w2) in SPL_D:
                            nc.tensor.matmul(ps[:, o2:o2 + w2],
                                             hT[:, t, 2 * p:2 * p + 2, :],
                                             wv_sb[:, 2 * p:2 * p + 2, o2:o2 + w2],
                                             start=(p == 0), stop=(p == 2),
                                             perf_mode=DR)
                    # even/odd head halves -> vpe/vpo fp8, descale 1/256
                    ev_out = bass.AP(tensor=vpe.tensor, offset=vpe.offset + t * 128,
                                     ap=[vpe.ap[0], [5 * 128, DCH], [1, 64]])
                    od_out = bass.AP(tensor=vpo.tensor, offset=vpo.offset + t * 128 + 64,
                                     ap=[vpo.ap[0], [5 * 128, DCH], [1, 64]])
                    ev_in = bass.AP(tensor=ps.tensor, offset=ps.offset,
                                    ap=[ps.ap[0], [128, DCH], [1, 64]])
                    od_in = bass.AP(tensor=ps.tensor, offset=ps.offset + 64,
                                    ap=[ps.ap[0], [128, DCH], [1, 64]])
                    nc.vector.tensor_scalar(out=ev_out[:tsz], in0=ev_in[:tsz],
                                            scalar1=IWS, scalar2=None, op0=ALU.mult)
                    nc.vector.tensor_scalar(out=od_out[:tsz], in0=od_in[:tsz],
                                            scalar1=IWS, scalar2=None, op0=ALU.mult)
                return hT

            def make_C_units(it, attn):
                """proj subunits, ln2 list, fc1 list, fc2 subunits for item it."""
                st = {}
                t0 = it * N

                def proj_u(t, o, tsz, o2, w2):
                    def f():
                        if f"r1_{t}" not in st:
                            st[f"r1_{t}"] = work.tile([128, D], BF16, name=f"r1_{it}_{t}",
                                                      tag=f"r1t{t}", bufs=2)
                            xr = work.tile([128, D], F32, name=f"xr_{it}_{t}",
                                           tag="xr", bufs=3)
                            nc.sync.dma_start(out=xr[:tsz, :], in_=x_d[t0 + o:t0 + o + tsz, :])
                            st[f"xr_{t}"] = xr
                        ps = psum.tile([128, 512], F32, name=f"pspj_{it}_{t}_{o2}",
                                       tag="small", bufs=2)
                        for kc in range(DCH):
                            nc.tensor.matmul(ps[:tsz, 0:w2],
                                             attn[:, kc, o:o + tsz],
                                             wproj_sb[:, kc, o2:o2 + w2],
                                             start=(kc == 0),
                                             stop=(kc == DCH - 1 and not use_bias_mm))
                        if use_bias_mm:
                            nc.tensor.matmul(ps[:tsz, 0:w2], ones_row[0:1, o:o + tsz],
                                             bprojr_sb[0:1, o2:o2 + w2],
                                             start=False, stop=True)
                        nc.vector.tensor_tensor(out=st[f"r1_{t}"][:tsz, o2:o2 + w2],
                                                in0=ps[:tsz, 0:w2],
                                                in1=st[f"xr_{t}"][:tsz, o2:o2 + w2],
                                                op=ALU.add)
                    return f

                def l2_stats(t, o, tsz):
                    def f():
                        if "st2" not in st:
                            st["st2"] = work.tile([128, 5, 3, 6], F32, name=f"st2_{it}",
                                                  tag="stats2", bufs=1)
                            st["mv2"] = work.tile([128, 5, 2], F32, name=f"mv2_{it}",
                                                  tag="mv2", bufs=1)
                            st["rs2"] = work.tile([128, 5], F32, name=f"rs2_{it}",
                                                  tag="rstd2", bufs=1)
                        ln_stats(st[f"r1_{t}"], st["st2"], st["mv2"], t, tsz, f"l2{it}")
                    return f

                def l2_newton():
                    newton_rsqrt(st["mv2"][:, :, 1], st["rs2"], f"l2{it}")

                def l2_apply(t, o, tsz):
                    def f():
                        ln_apply_tp(st[f"r1_{t}"], st["mv2"], st["rs2"], h2T, t, tsz,
                                    f"l2{it}", "dve")
                    return f

                def h2T_rhs03(p):
                    return bass.AP(tensor=h2T.tensor, offset=h2T.offset + 2 * p * 128,
                                   ap=[h2T.ap[0], [128, 2], [DCH * 128, 4], [1, 128]])

                def fc1_mc(mc):
                    def f():
                        ps = psum.tile([128, D], F32, name=f"psf1_{it}_{mc}", tag="big",
                                       bufs=3)
                        for p in range(3):
                            nc.tensor.matmul(ps[:, 0:512], wfc1_sb[:, mc, 2 * p:2 * p + 2, :],
                                             h2T_rhs03(p), start=(p == 0), stop=(p == 2),
                                             perf_mode=DR)
                        for p in range(3):
                            nc.tensor.matmul(ps[:, 512:640], wfc1_sb[:, mc, 2 * p:2 * p + 2, :],
                                             h2T[:, 4, 2 * p:2 * p + 2, :],
                                             start=(p == 0), stop=(p == 2), perf_mode=DR)
                        g2_out = bass.AP(tensor=g2.tensor, offset=g2.offset + mc * 128,
                                         ap=[g2.ap[0], [HCH * 128, 5], [1, 128]])
                        nc.scalar.activation(out=g2_out, in_=ps[:, 0:640], func=AF.Gelu,
                                             bias=bfc1_sb[:, mc:mc + 1], scale=IWS)
                    return f

                def fc2_u(t, o, tsz, o2, w2, last):
                    def f():
                        if "osb" not in st or st.get("osb_t") != t:
                            st["osb"] = work.tile([128, D], F32, name=f"osb_{it}_{t}",
                                                  tag="osb", bufs=3)
                            st["osb_t"] = t
                        ps = psum.tile([128, 512], F32, name=f"psf2_{it}_{t}_{o2}",
                                       tag="small", bufs=2)
                        for p in range(HCH // 2):
                            nc.tensor.matmul(ps[:, 0:w2],
                                             g2[:, t, 2 * p:2 * p + 2, :],
                                             wfc2_sb[:, 2 * p:2 * p + 2, o2:o2 + w2],
                                             start=(p == 0),
                                             stop=(p == HCH // 2 - 1 and not use_bias_mm),
                                             perf_mode=DR)
                        if use_bias_mm:
                            nc.tensor.matmul(ps[:, 0:w2], ones_row[0:1, 0:128],
                                             bfc2r_sb[0:1, o2:o2 + w2],
                                             start=False, stop=True)
                        nc.vector.scalar_tensor_tensor(
                            out=st["osb"][:tsz, o2:o2 + w2], in0=ps[:tsz, 0:w2],
                            scalar=IWS, in1=st[f"r1_{t}"][:tsz, o2:o2 + w2],
                            op0=ALU.mult, op1=ALU.add)
                        if last:
                            nc.sync.dma_start(out=out_d[t0 + o:t0 + o + tsz, :],
                                              in_=st["osb"][:tsz, :])
                    return f

                projs = []
                for t, (o, tsz) in enumerate(NT):
                    for (o2, w2) in SPL_D:
                        projs.append(proj_u(t, o, tsz, o2, w2))
                mid = [l2_stats(t, o, tsz) for t, (o, tsz) in enumerate(NT)]
                mid.append(l2_newton)
                mid += [l2_apply(t, o, tsz) for t, (o, tsz) in enumerate(NT)]
                mid += [fc1_mc(mc) for mc in range(HCH)]
                fc2s = []
                for t, (o, tsz) in enumerate(NT):
                    for j, (o2, w2) in enumerate(SPL_D):
                        fc2s.append(fc2_u(t, o, tsz, o2, w2, j == len(SPL_D) - 1))
                return projs, mid, fc2s

            def emit_B(it, hT, units_a, midblock, units_b):
                ua = list(units_a)
                ub = list(units_b)

                def unit(h):
                    lst = ua if h < 6 else ub
                    if lst:
                        lst.pop(0)()

                attn = work.tile([128, DCH, N], BF16, name=f"attn_{it}", tag="attnbuf",
                                 bufs=2)
                pend = [None]

                def flush_pair():
                    if pend[0] is None:
                        return
                    pv_e, pv_o, csb2, c = pend[0]
                    pend[0] = None
                    for (o, w) in SPL_N:
                        bc = psum.tile([128, 512], F32, name=f"bc_{it}_{c}_{o}",
                                       tag="small", bufs=2)
                        nc.tensor.matmul(bc[:, 0:w], selmat[64:65, 0, :],
                                         csb2[64:65, 0, o:o + w], start=True, stop=False)
                        nc.tensor.matmul(bc[:, 0:w], selmat[0:1, 1, :],
                                         csb2[0:1, 1, o:o + w], start=False, stop=True)
                        rec = work.tile([128, 512], F32, name=f"rec_{it}_{c}_{o}",
                                        tag="rec", bufs=2)
                        nc.vector.reciprocal_approx_fast(out=rec[:, 0:w], in_=bc[:, 0:w])
                        nc.vector.tensor_tensor(out=attn[0:64, c, o:o + w],
                                                in0=pv_e[0:64, o:o + w],
                                                in1=rec[0:64, 0:w], op=ALU.mult)
                        nc.vector.tensor_tensor(out=attn[64:128, c, o:o + w],
                                                in0=pv_o[64:128, o:o + w],
                                                in1=rec[64:128, 0:w], op=ALU.mult)

                pv_e_hold = [None]
                for h in range(H):
                    c = h // 2
                    base = 64 * (h % 2)
                    esall = work.tile([128, 5, N], F8, name=f"es_{it}_{h}", tag="esbuf",
                                      bufs=2)
                    for mt, (mo, msz) in enumerate(NT):
                        ss = psum.tile([128, D], F32, name=f"pss_{it}_{h}_{mt}",
                                       tag="big", bufs=3)
                        for (o, w) in SPL_N:
                            nc.tensor.matmul(ss[:, o:o + w],
                                             k_sb[base:base + 64, c, mo:mo + 128],
                                             q_sb[base:base + 64, c, o:o + w],
                                             start=True, stop=True)
                        if mt == 0:
                            flush_pair()
                        e_sb = work.tile([128, N], BF16, name=f"e_{it}_{h}_{mt}",
                                         tag="ebuf", bufs=4)
                        # GPSIMD can't read PSUM: route some tiles through an
                        # ACT psum->SBUF copy so Pool can do the mask multiply
                        if mt in (1, 3):
                            stmp = work.tile([128, N], BF16, name=f"sc_{it}_{h}_{mt}",
                                             tag="stmp", bufs=2)
                            nc.scalar.activation(out=stmp, in_=ss[:, 0:N], func=AF.Copy)
                            nc.gpsimd.tensor_tensor(out=e_sb, in0=stmp,
                                                    in1=maskt_sb[:, mt, :], op=ALU.mult)
                        else:
                            nc.vector.tensor_tensor(out=e_sb, in0=ss[:, 0:N],
                                                    in1=maskt_sb[:, mt, :], op=ALU.mult)
                        nc.scalar.activation(out=esall[:, mt, :], in_=e_sb, func=AF.Exp)
                    unit(h)
                    pv = psum.tile([128, D], F32, name=f"pspv_{it}_{h}", tag="big",
                                   bufs=3)
                    vp = vpe if h % 2 == 0 else vpo
                    for p in range(2):
                        for (o, w) in SPL_N:
                            rhs = bass.AP(tensor=esall.tensor,
                                          offset=esall.offset + 2 * p * N + o,
                                          ap=[esall.ap[0], [N, 2], [1, w]])
                            nc.tensor.matmul(pv[:, o:o + w], vp[:, c, 2 * p:2 * p + 2, :],
                                             rhs, start=(p == 0), stop=False,
                                             perf_mode=DR)
                    for (o, w) in SPL_N:
                        nc.tensor.matmul(pv[:, o:o + w], vp[:, c, 4, :],
                                         esall[:, 4, o:o + w], start=False, stop=True)
                    if h % 2 == 0:
                        csb2 = work.tile([128, 2, N], BF16, name=f"csb_{it}_{c}",
                                         tag="csbuf", bufs=2)
                        nc.scalar.activation(out=csb2[64:65, 0, :], in_=pv[64:65, 0:N],
                                             func=AF.Copy)
                        pv_e_hold[0] = (pv, csb2)
                    else:
                        pv_e, csb2 = pv_e_hold[0]
                        nc.scalar.activation(out=csb2[0:1, 1, :], in_=pv[0:1, 0:N],
                                             func=AF.Copy)
                        pend[0] = (pv_e, pv, csb2, c)
                    unit(h)
                    if h == 5:
                        flush_pair()
                        for u in ua:
                            u()
                        for u in midblock:
                            u()
                flush_pair()
                for u in ub:
                    u()
                return attn

            projs, mid, fc2s = [], [], []
            for it in range(IPC):
                hT = emit_A(it)
                if it == 0:
                    nc.sync.dma_start(out=wproj_sb, in_=wproj_d)
                    nc.sync.dma_start(out=wfc1_sb, in_=wfc1_d)
                    nc.sync.dma_start(out=wfc2_sb, in_=wfc2_d)
                attn = emit_B(it, hT, units_a=projs, midblock=mid, units_b=fc2s)
                projs, mid, fc2s = make_C_units(it, attn)
            for u in projs + mid + fc2s:
                u()

    nc.compile()
    return nc


def prep_in_maps(x, cp_mask, ln1_g, ln1_b, w_qkv, w_proj, b_proj,
                 ln2_g, ln2_b, w_fc1, b_fc1, w_fc2, b_fc2):
    bf = ml_dtypes.bfloat16
    e4 = ml_dtypes.float8_e4m3
    f = np.float32
    x = np.asarray(x, f)
    w_qkv = np.asarray(w_qkv, f)
    w_proj = np.asarray(w_proj, f)
    w_fc1 = np.asarray(w_fc1, f)
    w_fc2 = np.asarray(w_fc2, f)
    g1 = np.asarray(ln1_g, f)
    b1 = np.asarray(ln1_b, f)
    g2 = np.asarray(ln2_g, f)
    b2 = np.asarray(ln2_b, f)

    wqkv_eff = w_qkv * g1[:, None]
    bqkv = b1 @ w_qkv
    scale = DH ** -0.5

    def pair_layout(w):
        # [D, D] (k, m) -> [128, mc, kc, 128]
        return np.ascontiguousarray(
            w.reshape(DCH, 128, w.shape[1] // 128, 128).transpose(1, 2, 0, 3))

    wq = pair_layout(wqkv_eff[:, 0:D] * WS).astype(e4)
    wk = pair_layout(wqkv_eff[:, D:2 * D] * WS).astype(e4)
    wv = np.ascontiguousarray(
        (wqkv_eff[:, 2 * D:3 * D] * WS).reshape(DCH, 128, D).transpose(1, 0, 2)).astype(e4)
    bq = bqkv[0:D].astype(f)
    bk = bqkv[D:2 * D].astype(f)
    bv = bqkv[2 * D:3 * D]

    bprojr = (np.asarray(b_proj, f) + bv @ w_proj).astype(bf)
    wproj = np.ascontiguousarray(w_proj.reshape(DCH, 128, D).transpose(1, 0, 2)).astype(bf)

    wfc1_eff = (w_fc1 * g2[:, None]) * WS
    wfc1 = pair_layout(wfc1_eff).astype(e4)
    bfc1_eff = (np.asarray(b_fc1, f) + b2 @ w_fc1).astype(f)
    wfc2 = np.ascontiguousarray(
        (w_fc2 * WS).reshape(HCH, 128, D).transpose(1, 0, 2)).astype(e4)
    bfc2r = (np.asarray(b_fc2, f) * WS).astype(bf)

    maskt = np.ascontiguousarray(np.asarray(cp_mask, f)[0, 0].T * scale).astype(bf)
    xs = x.reshape(NCORES, TOK, D)

    shared = dict(maskt=maskt, wq=wq, wk=wk, wv=wv, bq=bq, bk=bk,
                  wproj=wproj, bprojr=bprojr,
                  wfc1=wfc1, bfc1=bfc1_eff,
                  wfc2=wfc2, bfc2r=bfc2r)
    return [dict(x=np.ascontiguousarray(xs[i]), **shared) for i in range(NCORES)]


_NC_CACHE = {}


def get_nc(use_bias_mm=True):
    key = ("nc", use_bias_mm)
    if key not in _NC_CACHE:
        _NC_CACHE[key] = build_nc(use_bias_mm=use_bias_mm)
    return _NC_CACHE[key]


def run(in_maps, trace=False, **kw):
    need_bias = bool(np.any(in_maps[0]["bprojr"].astype(np.float32))
                     or np.any(in_maps[0]["bfc2r"].astype(np.float32))
                     or np.any(in_maps[0]["bq"]) or np.any(in_maps[0]["bk"])
                     or np.any(in_maps[0]["bfc1"]))
    nc = get_nc(use_bias_mm=need_bias)
    return bass_utils.run_bass_kernel_spmd(nc, in_maps, core_ids=list(range(NCORES)),
                                           trace=trace, **kw)


def kernel(**inputs):
    in_maps = prep_in_maps(**inputs)
    res = run(in_maps)
    out = np.stack([res.results[i]["out"] for i in range(NCORES)])
    return out.reshape(B, N, D).astype(np.float32)


# revision 20
# speedup vs baseline: 1.4560x; 1.4560x over previous
"""Trainium2 Bass kernel for a ViT-style transformer block (sparse_attention).

v3: fp8 (e4m3) DoubleRow matmuls for qkv/pv/fc1/fc2 (K=256 per instruction,
2x+ PE throughput), bf16 scores/proj. Data-parallel over batch B=32 across
8 cores (4 items/core), no collectives.

Key structure per item:
  LN1 (DVE stats + Newton-rsqrt + apply) -> PE transpose -> hT fp8 (t-major
  [128, 5t, 6kc, 128] so DoubleRow stationary pairs are contiguous).
  q/k feature-major bf16 via fp8 DR matmuls (weights x256, descale 2^-8 in
  the PSUM->SBUF writeback); v -> fp8 vpe/vpo with ones-columns for softmax
  denominators. Scores via K=64 matmuls at base partition 0/64 (no zero
  padding); mask-mult split DVE/Pool; exp on ACT -> fp8; PV via DR pairs.
  Denominators broadcast per head-pair via a K=2 selector matmul, fast
  reciprocal, normalize on Pool -> attn bf16. proj bf16 + residual (Pool),
  LN2, fc1 DR + Gelu (descale via ACT scale) -> fp8, fc2 DR + fused
  descale+residual (scalar_tensor_tensor) -> out.

Work of item i-1 (proj/LN2/fc1/fc2) is interleaved into item i's head loop
to keep all engines busy (software pipeline from v2).
"""

import sys

sys.path.insert(0, "/opt/trn_rl_repo")

import numpy as np
import ml_dtypes

import concourse.bass as bass
import concourse.tile as tile
from concourse import bacc, mybir
from concourse import bass_utils
from concourse.masks import make_identity

F32 = mybir.dt.float32
BF16 = mybir.dt.bfloat16
F8 = mybir.dt.float8e4
I32 = mybir.dt.int32
DR = mybir.MatmulPerfMode.DoubleRow

B = 32
N = 577
D = 768
H = 12
DH = 64
HID = 3072
DCH = D // 128          # 6 chunks of the model dim
HCH = HID // 128        # 24 chunks of the hidden dim
NCORES = 8
IPC = B // NCORES       # items per core
TOK = IPC * N           # tokens per core

NT = [(0, 128), (128, 128), (256, 128), (384, 128), (512, 65)]
SPL_N = [(0, 512), (512, 65)]
SPL_D = [(0, 512), (512, 256)]
EPS = 1e-5
WS = 256.0              # fp8 weight scale
IWS = 1.0 / WS

AF = mybir.ActivationFunctionType
ALU = mybir.AluOpType


def build_nc(use_bias_mm=True):
    nc = bacc.Bacc("TRN2", target_bir_lowering=False, debug=False, num_devices=NCORES)

    x_d = nc.dram_tensor("x", [TOK, D], F32, kind="ExternalInput").ap()
    maskt_d = nc.dram_tensor("maskt", [N, N], BF16, kind="ExternalInput").ap()
    # wq/wk: [128, mc 6, kc 6, 128] fp8 (x256); wv: [128, kc 6, 768] fp8 (x256)
    wq_d = nc.dram_tensor("wq", [128, DCH, DCH, 128], F8, kind="ExternalInput").ap()
    wk_d = nc.dram_tensor("wk", [128, DCH, DCH, 128], F8, kind="ExternalInput").ap()
    wv_d = nc.dram_tensor("wv", [128, DCH, D], F8, kind="ExternalInput").ap()
    bq_d = nc.dram_tensor("bq", [D], F32, kind="ExternalInput").ap()
    bk_d = nc.dram_tensor("bk", [D], F32, kind="ExternalInput").ap()
    wproj_d = nc.dram_tensor("wproj", [128, DCH, D], BF16, kind="ExternalInput").ap()
    wfc1_d = nc.dram_tensor("wfc1", [128, HCH, DCH, 128], F8, kind="ExternalInput").ap()
    bfc1_d = nc.dram_tensor("bfc1", [HID], F32, kind="ExternalInput").ap()
    wfc2_d = nc.dram_tensor("wfc2", [128, HCH, D], F8, kind="ExternalInput").ap()
    if use_bias_mm:
        bprojr_d = nc.dram_tensor("bprojr", [D], BF16, kind="ExternalInput").ap()
        bfc2r_d = nc.dram_tensor("bfc2r", [D], BF16, kind="ExternalInput").ap()
    out_d = nc.dram_tensor("out", [TOK, D], F32, kind="ExternalOutput").ap()

    with tile.TileContext(nc) as tc:
        with (
            tc.tile_pool(name="const", bufs=1) as const,
            tc.tile_pool(name="work", bufs=1) as work,
            tc.tile_pool(name="psum", bufs=1, space="PSUM") as psum,
        ):
            # ---- constants / weights (resident) ----
            wq_sb = const.tile([128, DCH, DCH, 128], F8, name="wq_sb")
            nc.sync.dma_start(out=wq_sb, in_=wq_d)
            wk_sb = const.tile([128, DCH, DCH, 128], F8, name="wk_sb")
            nc.sync.dma_start(out=wk_sb, in_=wk_d)
            wv_sb = const.tile([128, DCH, D], F8, name="wv_sb")
            nc.sync.dma_start(out=wv_sb, in_=wv_d)
            wproj_sb = const.tile([128, DCH, D], BF16, name="wproj_sb")
            wfc1_sb = const.tile([128, HCH, DCH, 128], F8, name="wfc1_sb")
            wfc2_sb = const.tile([128, HCH, D], F8, name="wfc2_sb")

            bq_sb = const.tile([128, DCH], F32, name="bq_sb")
            nc.sync.dma_start(out=bq_sb, in_=bq_d.rearrange("(c p) -> p c", p=128))
            bk_sb = const.tile([128, DCH], F32, name="bk_sb")
            nc.sync.dma_start(out=bk_sb, in_=bk_d.rearrange("(c p) -> p c", p=128))
            bfc1_sb = const.tile([128, HCH], F32, name="bfc1_sb")
            nc.sync.dma_start(out=bfc1_sb, in_=bfc1_d.rearrange("(c p) -> p c", p=128))
            if use_bias_mm:
                bprojr_sb = const.tile([1, D], BF16, name="bprojr_sb")
                nc.sync.dma_start(out=bprojr_sb, in_=bprojr_d[None, :])
                bfc2r_sb = const.tile([1, D], BF16, name="bfc2r_sb")
                nc.sync.dma_start(out=bfc2r_sb, in_=bfc2r_d[None, :])
                ones_row = const.tile([1, N], BF16, name="ones_row")
                nc.vector.memset(ones_row, 1.0)

            maskt_sb = const.tile([128, 5, N], BF16, name="maskt_sb")
            nc.gpsimd.memset(maskt_sb[:, 4, :], 0.0)
            for mt, (mo, msz) in enumerate(NT):
                nc.sync.dma_start(out=maskt_sb[:msz, mt, :], in_=maskt_d[mo:mo + msz, :])

            ident = const.tile([128, 128], BF16, name="ident")
            make_identity(nc, ident)
            # selector rows for pair-denominator broadcast. The even-head
            # denominator sits on partition 64 (ones col 64), the odd-head
            # one on partition 0; each selector row lives on the matching
            # partition: slot 0 @ p64 spreads even -> partitions 0:64,
            # slot 1 @ p0 spreads odd -> partitions 64:128.
            selmat = const.tile([128, 2, 128], BF16, name="selmat")
            nc.gpsimd.memset(selmat[0:1], 0.0)
            nc.gpsimd.memset(selmat[64:65], 0.0)
            nc.gpsimd.memset(selmat[64:65, 0, 0:64], 1.0)
            nc.gpsimd.memset(selmat[0:1, 1, 64:128], 1.0)

            # v stationary tiles [128 keys, c 6, mt 5, 128] fp8
            # vpe: cols 0:64 = even-head dims, col 64 = ones (denom), 65:128 = 0
            # vpo: col 0 = ones (denom), 1:64 = 0, cols 64:128 = odd-head dims
            vpe = const.tile([128, DCH, 5, 128], F8, name="vpe")
            vpo = const.tile([128, DCH, 5, 128], F8, name="vpo")
            nc.gpsimd.memset(vpe, 0.0)
            nc.gpsimd.memset(vpo, 0.0)
            for mt, (mo, msz) in enumerate(NT):
                nc.gpsimd.memset(vpe[0:msz, :, mt, 64:65], 1.0)
                nc.gpsimd.memset(vpo[0:msz, :, mt, 0:1], 1.0)

            # persistent per-item buffers (stable addresses via fixed tags)
            k_sb = const.tile([128, DCH, 640], BF16, name="k_sb")
            nc.gpsimd.memset(k_sb, 0.0)
            q_sb = const.tile([128, DCH, N], BF16, name="q_sb")
            hT_t = [const.tile([128, 5, DCH, 128], F8, name=f"hT{i}") for i in range(2)]
            for t_ in hT_t:
                nc.gpsimd.memset(t_[:, 4, :, 65:128], 0.0)
            h2T = const.tile([128, 5, DCH, 128], F8, name="h2T")
            nc.gpsimd.memset(h2T[:, 4, :, 65:128], 0.0)
            g2 = const.tile([128, 5, HCH, 128], F8, name="g2")

            def newton_rsqrt(var_ap, rstd, pfx):
                """rstd[:,0:5] = (var_ap + EPS) ** -0.5, DVE only."""
                ve = work.tile([128, 5], F32, name=f"ve_{pfx}", tag="nwt_ve", bufs=2)
                nc.vector.tensor_scalar(out=ve, in0=var_ap, scalar1=EPS, scalar2=None,
                                        op0=ALU.add)
                yi = work.tile([128, 5], I32, name=f"yi_{pfx}", tag="nwt_yi", bufs=2)
                nc.vector.tensor_scalar(out=yi, in0=ve.bitcast(I32), scalar1=1,
                                        scalar2=None, op0=ALU.logical_shift_right)
                nc.vector.tensor_scalar(out=yi, in0=yi, scalar1=-1, scalar2=None,
                                        op0=ALU.bitwise_xor)
                nc.vector.tensor_scalar(out=yi, in0=yi, scalar1=0x5f3759e0,
                                        scalar2=None, op0=ALU.add)
                y = yi.bitcast(F32)
                t1 = work.tile([128, 5], F32, name=f"t1_{pfx}", tag="nwt_t1", bufs=2)
                for _ in range(2):
                    nc.vector.tensor_tensor(out=t1, in0=y, in1=y, op=ALU.mult)
                    nc.vector.tensor_tensor(out=t1, in0=t1, in1=ve, op=ALU.mult)
                    nc.vector.tensor_scalar(out=t1, in0=t1, scalar1=-0.5, scalar2=1.5,
                                            op0=ALU.mult, op1=ALU.add)
                    nc.vector.tensor_tensor(out=y, in0=y, in1=t1, op=ALU.mult)
                nc.vector.tensor_copy(out=rstd, in_=y)

            def ln_stats(src, statsall, mvall, t, tsz, pfx):
                nc.vector.bn_stats(out=statsall[:tsz, t, 0, :], in_=src[:tsz, 0:256])
                nc.vector.bn_stats(out=statsall[:tsz, t, 1, :], in_=src[:tsz, 256:512])
                nc.vector.bn_stats(out=statsall[:tsz, t, 2, :], in_=src[:tsz, 512:768])
                nc.vector.bn_aggr(out=mvall[:tsz, t, :], in_=statsall[:tsz, t])

            def ln_apply_tp(src, mvall, rstd, dst, t, tsz, pfx, cp_eng):
                """(src - mean) * rstd -> bf16 -> PE transpose -> dst[:, t] fp8."""
                htm = work.tile([128, D], BF16, name=f"htm_{pfx}_{t}", tag="htm", bufs=2)
                nc.vector.tensor_scalar(out=htm[:tsz], in0=src[:tsz],
                                        scalar1=mvall[:tsz, t, 0:1],
                                        scalar2=rstd[:tsz, t:t + 1],
                                        op0=ALU.subtract, op1=ALU.mult)
                tp = psum.tile([128, D], BF16, name=f"tp_{pfx}_{t}", tag="small", bufs=2)
                for c in range(DCH):
                    nc.tensor.transpose(tp[:, c * 128:c * 128 + tsz],
                                        htm[:tsz, c * 128:(c + 1) * 128],
                                        ident[:tsz, :tsz])
                dst_ap = bass.AP(tensor=dst.tensor, offset=dst.offset + t * DCH * 128,
                                 ap=[dst.ap[0], [128, DCH], [1, tsz]])
                src_ap = tp.rearrange("p (c q) -> p c q", c=DCH)[:, :, :tsz]
                if cp_eng == "act":
                    nc.scalar.activation(out=dst_ap, in_=src_ap, func=AF.Copy)
                else:
                    nc.vector.tensor_copy(out=dst_ap, in_=src_ap)

            def emit_A(it):
                """x load, LN1, hT fp8, q/k bf16, v fp8 for item `it`."""
                t0 = it * N
                hT = hT_t[it % 2]
                xall = work.tile([128, 5, D], F32, name=f"xall_{it}", tag="xall", bufs=1)
                statsall = work.tile([128, 5, 3, 6], F32, name=f"st1_{it}",
                                     tag="stats1", bufs=1)
                mvall = work.tile([128, 5, 2], F32, name=f"mv1_{it}", tag="mv1", bufs=1)
                rstd = work.tile([128, 5], F32, name=f"rs1_{it}", tag="rstd1", bufs=1)
                for t, (o, tsz) in enumerate(NT):
                    nc.sync.dma_start(out=xall[:tsz, t, :], in_=x_d[t0 + o:t0 + o + tsz, :])
                    ln_stats(xall[:, t, :], statsall, mvall, t, tsz, f"a{it}")
                newton_rsqrt(mvall[:, :, 1], rstd, f"a{it}")
                for t, (o, tsz) in enumerate(NT):
                    ln_apply_tp(xall[:, t, :], mvall, rstd, hT, t, tsz, f"a{it}", "act")

                # rhs moving views per pair: t0..t3 (512 cols) + t4 (128 cols)
                def hT_rhs03(hh, p):
                    return bass.AP(tensor=hh.tensor, offset=hh.offset + 2 * p * 128,
                                   ap=[hh.ap[0], [128, 2], [DCH * 128, 4], [1, 128]])

                def qk_chunk(ps, w_sb_, mc):
                    for p in range(3):
                        nc.tensor.matmul(ps[:, 0:512], w_sb_[:, mc, 2 * p:2 * p + 2, :],
                                         hT_rhs03(hT, p), start=(p == 0), stop=(p == 2),
                                         perf_mode=DR)
                    for p in range(3):
                        nc.tensor.matmul(ps[:, 512:640], w_sb_[:, mc, 2 * p:2 * p + 2, :],
                                         hT[:, 4, 2 * p:2 * p + 2, :],
                                         start=(p == 0), stop=(p == 2), perf_mode=DR)

                def wb(dst, ps, b_sb, mc):
                    if use_bias_mm:
                        nc.vector.tensor_scalar(out=dst, in0=ps[:, 0:N], scalar1=IWS,
                                                scalar2=b_sb[:, mc:mc + 1],
                                                op0=ALU.mult, op1=ALU.add)
                    else:
                        nc.vector.tensor_scalar(out=dst, in0=ps[:, 0:N], scalar1=IWS,
                                                scalar2=None, op0=ALU.mult)

                for mc in range(DCH):
                    ps = psum.tile([128, D], F32, name=f"psq_{it}_{mc}", tag="big", bufs=3)
                    qk_chunk(ps, wq_sb, mc)
                    wb(q_sb[:, mc, :], ps, bq_sb, mc)
                for mc in range(DCH):
                    ps = psum.tile([128, D], F32, name=f"psk_{it}_{mc}", tag="big", bufs=3)
                    qk_chunk(ps, wk_sb, mc)
                    wb(k_sb[:, mc, 0:N], ps, bk_sb, mc)
                for t, (o, tsz) in enumerate(NT):
                    ps = psum.tile([128, D], F32, name=f"psv_{it}_{t}", tag="big", bufs=3)
                    for p in range(3):
                        for (o2, w2) in SPL_D:
                            nc.tensor.matmul(ps[:, o2:o2 + w2],
                                             hT[:, t, 2 * p:2 * p + 2, :],
                                             wv_sb[:, 2 * p:2 * p + 2, o2:o2 + w2],
                                             start=(p == 0), stop=(p == 2),
                                             perf_mode=DR)
                    # even/odd head halves -> vpe/vpo fp8, descale 1/256
                    ev_out = bass.AP(tensor=vpe.tensor, offset=vpe.offset + t * 128,
                                     ap=[vpe.ap[0], [5 * 128, DCH], [1, 64]])
                    od_out = bass.AP(tensor=vpo.tensor, offset=vpo.offset + t * 128 + 64,
                                     ap=[vpo.ap[0], [5 * 128, DCH], [1, 64]])
                    ev_in = bass.AP(tensor=ps.tensor, offset=ps.offset,
                                    ap=[ps.ap[0], [128, DCH], [1, 64]])
                    od_in = bass.AP(tensor=ps.tensor, offset=ps.offset + 64,
                                    ap=[ps.ap[0], [128, DCH], [1, 64]])
                    nc.vector.tensor_scalar(out=ev_out[:tsz], in0=ev_in[:tsz],
                                            scalar1=IWS, scalar2=None, op0=ALU.mult)
                    nc.vector.tensor_scalar(out=od_out[:tsz], in0=od_in[:tsz],
                                            scalar1=IWS, scalar2=None, op0=ALU.mult)
                return hT

            def make_C_units(it, attn):
                """proj subunits, ln2 list, fc1 list, fc2 subunits for item it."""
                st = {}
                t0 = it * N

                def proj_u(t, o, tsz, o2, w2):
                    def f():
                        if f"r1_{t}" not in st:
                            st[f"r1_{t}"] = work.tile([128, D], BF16, name=f"r1_{it}_{t}",
                                                      tag=f"r1t{t}", bufs=2)
                            xr = work.tile([128, D], F32, name=f"xr_{it}_{t}",
                                           tag="xr", bufs=3)
                            nc.sync.dma_start(out=xr[:tsz, :], in_=x_d[t0 + o:t0 + o + tsz, :])
                            st[f"xr_{t}"] = xr
                        ps = psum.tile([128, 512], F32, name=f"pspj_{it}_{t}_{o2}",
                                       tag="small", bufs=2)
                        for kc in range(DCH):
                            nc.tensor.matmul(ps[:tsz, 0:w2],
                                             attn[:, kc, o:o + tsz],
                                             wproj_sb[:, kc, o2:o2 + w2],
                                             start=(kc == 0),
                                             stop=(kc == DCH - 1 and not use_bias_mm))
                        if use_bias_mm:
                            nc.tensor.matmul(ps[:tsz, 0:w2], ones_row[0:1, o:o + tsz],
                                             bprojr_sb[0:1, o2:o2 + w2],
                                             start=False, stop=True)
                        nc.vector.tensor_tensor(out=st[f"r1_{t}"][:tsz, o2:o2 + w2],
                                                in0=ps[:tsz, 0:w2],
                                                in1=st[f"xr_{t}"][:tsz, o2:o2 + w2],
                                                op=ALU.add)
                    return f

                def l2_stats(t, o, tsz):
                    def f():
                        if "st2" not in st:
                            st["st2"] = work.tile([128, 5, 3, 6], F32, name=f"st2_{it}",
                                                  tag="stats2", bufs=1)
                            st["mv2"] = work.tile([128, 5, 2], F32, name=f"mv2_{it}",
                                                  tag="mv2", bufs=1)
                            st["rs2"] = work.tile([128, 5], F32, name=f"rs2_{it}",
                                                  tag="rstd2", bufs=1)
                        ln_stats(st[f"r1_{t}"], st["st2"], st["mv2"], t, tsz, f"l2{it}")
                    return f

                def l2_newton():
                    newton_rsqrt(st["mv2"][:, :, 1], st["rs2"], f"l2{it}")

                def l2_apply(t, o, tsz):
                    def f():
                        ln_apply_tp(st[f"r1_{t}"], st["mv2"], st["rs2"], h2T, t, tsz,
                                    f"l2{it}", "dve")
                    return f

                def h2T_rhs03(p):
                    return bass.AP(tensor=h2T.tensor, offset=h2T.offset + 2 * p * 128,
                                   ap=[h2T.ap[0], [128, 2], [DCH * 128, 4], [1, 128]])

                def fc1_mc(mc):
                    def f():
                        ps = psum.tile([128, D], F32, name=f"psf1_{it}_{mc}", tag="big",
                                       bufs=3)
                        for p in range(3):
                            nc.tensor.matmul(ps[:, 0:512], wfc1_sb[:, mc, 2 * p:2 * p + 2, :],
                                             h2T_rhs03(p), start=(p == 0), stop=(p == 2),
                                             perf_mode=DR)
                        for p in range(3):
                            nc.tensor.matmul(ps[:, 512:640], wfc1_sb[:, mc, 2 * p:2 * p + 2, :],
                                             h2T[:, 4, 2 * p:2 * p + 2, :],
                                             start=(p == 0), stop=(p == 2), perf_mode=DR)
                        g2_out = bass.AP(tensor=g2.tensor, offset=g2.offset + mc * 128,
                                         ap=[g2.ap[0], [HCH * 128, 5], [1, 128]])
                        nc.scalar.activation(out=g2_out, in_=ps[:, 0:640], func=AF.Gelu,
                                             bias=bfc1_sb[:, mc:mc + 1], scale=IWS)
                    return f

                def fc2_u(t, o, tsz, o2, w2, last):
                    def f():
                        if "osb" not in st or st.get("osb_t") != t:
                            st["osb"] = work.tile([128, D], F32, name=f"osb_{it}_{t}",
                                                  tag="osb", bufs=3)
                            st["osb_t"] = t
                        ps = psum.tile([128, 512], F32, name=f"psf2_{it}_{t}_{o2}",
                                       tag="small", bufs=2)
                        for p in range(HCH // 2):
                            nc.tensor.matmul(ps[:, 0:w2],
                                             g2[:, t, 2 * p:2 * p + 2, :],
                                             wfc2_sb[:, 2 * p:2 * p + 2, o2:o2 + w2],
                                             start=(p == 0),
                                             stop=(p == HCH // 2 - 1 and not use_bias_mm),
                                             perf_mode=DR)
                        if use_bias_mm:
                            nc.tensor.matmul(ps[:, 0:w2], ones_row[0:1, 0:128],
                                             bfc2r_sb[0:1, o2:o2 + w2],
                                             start=False, stop=True)
                        nc.vector.scalar_tensor_tensor(
                            out=st["osb"][:tsz, o2:o2 + w2], in0=ps[:tsz, 0:w2],
                            scalar=IWS, in1=st[f"r1_{t}"][:tsz, o2:o2 + w2],
                            op0=ALU.mult, op1=ALU.add)
                        if last:
                            nc.sync.dma_start(out=out_d[t0 + o:t0 + o + tsz, :],
                                              in_=st["osb"][:tsz, :])
                    return f

                projs = []
                for t, (o, tsz) in enumerate(NT):
                    for (o2, w2) in SPL_D:
                        projs.append(proj_u(t, o, tsz, o2, w2))
                mid = [l2_stats(t, o, tsz) for t, (o, tsz) in enumerate(NT)]
                mid.append(l2_newton)
                mid += [l2_apply(t, o, tsz) for t, (o, tsz) in enumerate(NT)]
                mid += [fc1_mc(mc) for mc in range(HCH)]
                fc2s = []
                for t, (o, tsz) in enumerate(NT):
                    for j, (o2, w2) in enumerate(SPL_D):
                        fc2s.append(fc2_u(t, o, tsz, o2, w2, j == len(SPL_D) - 1))
                return projs, mid, fc2s

            def emit_B(it, hT, units_a, midblock, units_b):
                ua = list(units_a)
                ub = list(units_b)

                def unit(h):
                    lst = ua if h < 6 else ub
                    if lst:
                        lst.pop(0)()

                attn = work.tile([128, DCH, N], BF16, name=f"attn_{it}", tag="attnbuf",
                                 bufs=2)
                pend = [None]

                def flush_pair():
                    if pend[0] is None:
                        return
                    pv_e, pv_o, csb2, c = pend[0]
                    pend[0] = None
                    for (o, w) in SPL_N:
                        bc = psum.tile([128, 512], F32, name=f"bc_{it}_{c}_{o}",
                                       tag="small", bufs=2)
                        nc.tensor.matmul(bc[:, 0:w], selmat[64:65, 0, :],
                                         csb2[64:65, 0, o:o + w], start=True, stop=False)
                        nc.tensor.matmul(bc[:, 0:w], selmat[0:1, 1, :],
                                         csb2[0:1, 1, o:o + w], start=False, stop=True)
                        rec = work.tile([128, 512], F32, name=f"rec_{it}_{c}_{o}",
                                        tag="rec", bufs=2)
                        nc.vector.reciprocal_approx_fast(out=rec[:, 0:w], in_=bc[:, 0:w])
                        nc.vector.tensor_tensor(out=attn[0:64, c, o:o + w],
                                                in0=pv_e[0:64, o:o + w],
                                                in1=rec[0:64, 0:w], op=ALU.mult)
                        nc.vector.tensor_tensor(out=attn[64:128, c, o:o + w],
                                                in0=pv_o[64:128, o:o + w],
                                                in1=rec[64:128, 0:w], op=ALU.mult)

                pv_e_hold = [None]
                for h in range(H):
                    c = h // 2
                    base = 64 * (h % 2)
                    esall = work.tile([128, 5, N], F8, name=f"es_{it}_{h}", tag="esbuf",
                                      bufs=2)
                    for mt, (mo, msz) in enumerate(NT):
                        ss = psum.tile([128, D], F32, name=f"pss_{it}_{h}_{mt}",
                                       tag="big", bufs=3)
                        for (o, w) in SPL_N:
                            nc.tensor.matmul(ss[:, o:o + w],
                                             k_sb[base:base + 64, c, mo:mo + 128],
                                             q_sb[base:base + 64, c, o:o + w],
                                             start=True, stop=True)
                        if mt == 0:
                            flush_pair()
                        e_sb = work.tile([128, N], BF16, name=f"e_{it}_{h}_{mt}",
                                         tag="ebuf", bufs=4)
                        # GPSIMD can't read PSUM: route some tiles through an
                        # ACT psum->SBUF copy so Pool can do the mask multiply
                        if mt in (1, 3):
                            stmp = work.tile([128, N], BF16, name=f"sc_{it}_{h}_{mt}",
                                             tag="stmp", bufs=2)
                            nc.scalar.activation(out=stmp, in_=ss[:, 0:N], func=AF.Copy)
                            nc.gpsimd.tensor_tensor(out=e_sb, in0=stmp,
                                                    in1=maskt_sb[:, mt, :], op=ALU.mult)
                        else:
                            nc.vector.tensor_tensor(out=e_sb, in0=ss[:, 0:N],
                                                    in1=maskt_sb[:, mt, :], op=ALU.mult)
                        nc.scalar.activation(out=esall[:, mt, :], in_=e_sb, func=AF.Exp)
                    unit(h)
                    pv = psum.tile([128, D], F32, name=f"pspv_{it}_{h}", tag="big",
                                   bufs=3)
                    vp = vpe if h % 2 == 0 else vpo
                    for p in range(2):
                        for (o, w) in SPL_N:
                            rhs = bass.AP(tensor=esall.tensor,
                                          offset=esall.offset + 2 * p * N + o,
                                          ap=[esall.ap[0], [N, 2], [1, w]])
                            nc.tensor.matmul(pv[:, o:o + w], vp[:, c, 2 * p:2 * p + 2, :],
                                             rhs, start=(p == 0), stop=False,
                                             perf_mode=DR)
                    for (o, w) in SPL_N:
                        nc.tensor.matmul(pv[:, o:o + w], vp[:, c, 4, :],
                                         esall[:, 4, o:o + w], start=False, stop=True)
                    if h % 2 == 0:
                        csb2 = work.tile([128, 2, N], BF16, name=f"csb_{it}_{c}",
                                         tag="csbuf", bufs=2)
                        nc.scalar.activation(out=csb2[64:65, 0, :], in_=pv[64:65, 0:N],
                                             func=AF.Copy)
                        pv_e_hold[0] = (pv, csb2)
                    else:
                        pv_e, csb2 = pv_e_hold[0]
                        nc.scalar.activation(out=csb2[0:1, 1, :], in_=pv[0:1, 0:N],
                                             func=AF.Copy)
                        pend[0] = (pv_e, pv, csb2, c)
                    unit(h)
                    if h == 5:
                        flush_pair()
                        for u in ua:
                            u()
                        for u in midblock:
                            u()
                flush_pair()
                for u in ub:
                    u()
                return attn

            projs, mid, fc2s = [], [], []
            for it in range(IPC):
                hT = emit_A(it)
                if it == 0:
                    nc.sync.dma_start(out=wproj_sb, in_=wproj_d)
                    nc.sync.dma_start(out=wfc1_sb, in_=wfc1_d)
                    nc.sync.dma_start(out=wfc2_sb, in_=wfc2_d)
                attn = emit_B(it, hT, units_a=projs, midblock=mid, units_b=fc2s)
                projs, mid, fc2s = make_C_units(it, attn)
            for u in projs + mid + fc2s:
                u()

    nc.compile()
    return nc


def prep_in_maps(x, cp_mask, ln1_g, ln1_b, w_qkv, w_proj, b_proj,
                 ln2_g, ln2_b, w_fc1, b_fc1, w_fc2, b_fc2):
    bf = ml_dtypes.bfloat16
    e4 = ml_dtypes.float8_e4m3
    f = np.float32
    x = np.asarray(x, f)
    w_qkv = np.asarray(w_qkv, f)
    w_proj = np.asarray(w_proj, f)
    w_fc1 = np.asarray(w_fc1, f)
    w_fc2 = np.asarray(w_fc2, f)
    g1 = np.asarray(ln1_g, f)
    b1 = np.asarray(ln1_b, f)
    g2 = np.asarray(ln2_g, f)
    b2 = np.asarray(ln2_b, f)

    wqkv_eff = w_qkv * g1[:, None]
    bqkv = b1 @ w_qkv
    scale = DH ** -0.5

    def pair_layout(w):
        # [D, D] (k, m) -> [128, mc, kc, 128]
        return np.ascontiguousarray(
            w.reshape(DCH, 128, w.shape[1] // 128, 128).transpose(1, 2, 0, 3))

    wq = pair_layout(wqkv_eff[:, 0:D] * WS).astype(e4)
    wk = pair_layout(wqkv_eff[:, D:2 * D] * WS).astype(e4)
    wv = np.ascontiguousarray(
        (wqkv_eff[:, 2 * D:3 * D] * WS).reshape(DCH, 128, D).transpose(1, 0, 2)).astype(e4)
    bq = bqkv[0:D].astype(f)
    bk = bqkv[D:2 * D].astype(f)
    bv = bqkv[2 * D:3 * D]

    bprojr = (np.asarray(b_proj, f) + bv @ w_proj).astype(bf)
    wproj = np.ascontiguousarray(w_proj.reshape(DCH, 128, D).transpose(1, 0, 2)).astype(bf)

    wfc1_eff = (w_fc1 * g2[:, None]) * WS
    wfc1 = pair_layout(wfc1_eff).astype(e4)
    bfc1_eff = (np.asarray(b_fc1, f) + b2 @ w_fc1).astype(f)
    wfc2 = np.ascontiguousarray(
        (w_fc2 * WS).reshape(HCH, 128, D).transpose(1, 0, 2)).astype(e4)
    bfc2r = (np.asarray(b_fc2, f) * WS).astype(bf)

    maskt = np.ascontiguousarray(np.asarray(cp_mask, f)[0, 0].T * scale).astype(bf)
    xs = x.reshape(NCORES, TOK, D)

    shared = dict(maskt=maskt, wq=wq, wk=wk, wv=wv, bq=bq, bk=bk,
                  wproj=wproj, bprojr=bprojr,
                  wfc1=wfc1, bfc1=bfc1_eff,
                  wfc2=wfc2, bfc2r=bfc2r)
    return [dict(x=np.ascontiguousarray(xs[i]), **shared) for i in range(NCORES)]


_NC_CACHE = {}


def get_nc(use_bias_mm=True):
    key = ("nc", use_bias_mm)
    if key not in _NC_CACHE:
        _NC_CACHE[key] = build_nc(use_bias_mm=use_bias_mm)
    return _NC_CACHE[key]


def run(in_maps, trace=False, **kw):
    need_bias = bool(np.any(in_maps[0]["bprojr"].astype(np.float32))
                     or np.any(in_maps[0]["bfc2r"].astype(np.float32))
                     or np.any(in_maps[0]["bq"]) or np.any(in_maps[0]["bk"])
                     or np.any(in_maps[0]["bfc1"]))
    nc = get_nc(use_bias_mm=need_bias)
    return bass_utils.run_bass_kernel_spmd(nc, in_maps, core_ids=list(range(NCORES)),
                                           trace=trace, **kw)


def kernel(**inputs):
    in_maps = prep_in_maps(**inputs)
    res = run(in_maps)
    out = np.stack([res.results[i]["out"] for i in range(NCORES)])
    return out.reshape(B, N, D).astype(np.float32)
